# revision 56
# baseline (speedup 1.0000x reference)
"""GRU4Rec Trainium2 kernel: 8-core SPMD, latency-optimized recurrence.

Sharding: data-parallel over batch (32 seqs/core); vocab-sharded tied-embedding
logits with an on-device AllGather of the final hidden state.

Recurrence critical path per step (the serial cycle):
  PE r-gate matmuls -> ACT sigmoid(r) -> DVE qs,q2 -> ACT tanh -> DVE ws,hb -> PE
All other work is pushed off that cycle:
  - x-side gates, b_hh(g) injected into PSUM by PE identity-matmuls, emitted
    ahead of the weight matmuls so they run during the previous step's EW.
  - sigma(z), zc=1-z, us=z*h run during the tanh window.
  - phase-1 chunk work (gather/transpose/gates for t+16..) is emitted one
    piece per step, alternating ACT/DVE for the psum->sbuf copies.
PSUM slot order [r0, r1, z0, z1] lets sigmoid(r) fire after only 4 weight MMs.
"""

import numpy as np
import ml_dtypes

B, T, H, V = 256, 200, 256, 50000
NCORES = 8
BL = B // NCORES          # 32 sequences per core
NTOK = BL * T             # 6400 tokens per core
VS = 6250                 # vocab stride per core
VSC = VS + 1              # per-core logits width (overlap of 1)
CHUNK_T = 8               # timesteps per phase-1 chunk
NCHUNK = T // CHUNK_T     # 25
CTOK = BL * CHUNK_T       # 256 tokens per chunk
BIGMASK = 60.0            # sigmoid(x + 60) == 1.0 in fp32

_cache = {}


def _build_nc(debug=False):
    import concourse.bass as bass
    import concourse.mybir as mybir
    import concourse.tile as tile
    from concourse import bacc
    from concourse.bass import IndirectOffsetOnAxis

    f32 = mybir.dt.float32
    bf16 = mybir.dt.bfloat16
    i32 = mybir.dt.int32
    AF = mybir.ActivationFunctionType
    OP = mybir.AluOpType

    nc = bacc.Bacc(None, target_bir_lowering=False, debug=False, num_devices=NCORES)

    ids_d = nc.dram_tensor("ids", [NTOK, 1], i32, kind="ExternalInput")
    maskr_d = nc.dram_tensor("maskrow", [1, NTOK], bf16, kind="ExternalInput")
    emb_d = nc.dram_tensor("emb", [V + 1, H], f32, kind="ExternalInput")
    embt_d = nc.dram_tensor("embt", [H, VSC], bf16, kind="ExternalInput")
    wih_d = nc.dram_tensor("wih", [H, 3 * H], bf16, kind="ExternalInput")
    whh_d = nc.dram_tensor("whh", [H, 3 * H], bf16, kind="ExternalInput")
    biasf_d = nc.dram_tensor("biasf", [3 * H, 1], f32, kind="ExternalInput")
    idm_d = nc.dram_tensor("idm", [128, 128], f32, kind="ExternalInput")
    bhhg_d = nc.dram_tensor("bhhg", [H, 1], f32, kind="ExternalInput")
    out_d = nc.dram_tensor("out", [B, VSC], bf16, kind="ExternalOutput")

    # weight-column m-slice -> psum slot: weights are [z0 z1 r0 r1 g0 g1],
    # psum zr-slots are [r0 r1 z0 z1]
    M2SLOT = {0: 2, 1: 3, 2: 0, 3: 1}

    with tile.TileContext(nc) as tc:
        with (
            tc.tile_pool(name="const", bufs=1) as const,
            tc.tile_pool(name="gstore", bufs=1) as gstore,
            tc.tile_pool(name="gin", bufs=6) as gin,
            tc.tile_pool(name="xtp", bufs=4) as xtp,
            tc.tile_pool(name="ew", bufs=3) as ew,
            tc.tile_pool(name="hst", bufs=3) as hst,
            tc.tile_pool(name="lop", bufs=2) as lop,
            tc.tile_pool(name="ldram", bufs=1, space="DRAM") as ldram,
            tc.tile_pool(name="ptr", bufs=1, space="PSUM") as ptr,
            tc.tile_pool(name="pg1", bufs=4, space="PSUM") as pg1,
            tc.tile_pool(name="pzr", bufs=2, space="PSUM") as pzr,
            tc.tile_pool(name="pgg", bufs=1, space="PSUM") as pgg,
        ):
            # ---- constants / weights ----
            wih_sb = const.tile([128, 2 * 3 * H], bf16)
            whh_sb = const.tile([128, 2 * 3 * H], bf16)
            ident_f = const.tile([128, 128], f32)
            nc.sync.dma_start(ident_f[:, :], idm_d[:, :])

            # ids for the 4 startup chunks go out before the bulk weight DMAs
            gin_pool = gin

            def p1_ids(c):
                tok0 = c * CTOK
                idts = []
                for tt in range(2):
                    idt = gin_pool.tile([128, 1], i32, tag="idt", name="idt")
                    nc.sync.dma_start(
                        idt[:, :],
                        ids_d[tok0 + tt * 128: tok0 + (tt + 1) * 128, :])
                    idts.append(idt)
                return idts

            start_idts = {0: p1_ids(0)}

            for k in range(2):
                nc.sync.dma_start(wih_sb[:, k * 768:(k + 1) * 768],
                                  wih_d[k * 128:(k + 1) * 128, :])
            start_idts.update({c: p1_ids(c) for c in range(1, 4)})
            for k in range(2):
                nc.sync.dma_start(whh_sb[:, k * 768:(k + 1) * 768],
                                  whh_d[k * 128:(k + 1) * 128, :])
            ident = const.tile([128, 128], bf16)
            nc.vector.tensor_copy(ident[:, :], ident_f[:, :])
            wbig = const.tile([1, 128], bf16)
            nc.vector.memset(wbig[:, :], BIGMASK)
            maskr = const.tile([1, NTOK], bf16)
            nc.sync.dma_start(maskr[:, :], maskr_d[:, :])
            bias_sb = const.tile([128, 6], f32)
            nc.sync.dma_start(bias_sb[:, :],
                              biasf_d.rearrange("(m p) o -> p (m o)", p=128))
            bhhg_sb = const.tile([128, 2], f32)
            nc.sync.dma_start(bhhg_sb[:, :],
                              bhhg_d.rearrange("(g p) o -> p (g o)", p=128))
            bhhg_bc = const.tile([128, 2, 32], bf16)
            for g in range(2):
                nc.vector.tensor_copy(bhhg_bc[:, g, :],
                                      bhhg_sb[:, g:g + 1].to_broadcast([128, 32]))

            # persistent x-side gate stores
            gzr = gstore.tile([128, T, 4, 32], bf16)   # slots [r0 r1 z0 z1]
            gg = gstore.tile([128, T, 2, 32], bf16)    # candidate x-gates

            def wih_sl(k, m):
                return wih_sb[:, k * 768 + m * 128: k * 768 + (m + 1) * 128]

            def whh_sl(k, m):
                return whh_sb[:, k * 768 + m * 128: k * 768 + (m + 1) * 128]

            # ---- phase-1 chunk pieces ----
            # xtc: [H-part, k, token] f32-gathered -> transposed -> bf16
            def p1_gather(c):
                idts = start_idts.pop(c, None) or p1_ids(c)
                xgs = []
                for tt in range(2):
                    xg = gin.tile([128, H], f32, tag=f"xg{tt}")
                    nc.gpsimd.indirect_dma_start(
                        out=xg[:, :], out_offset=None, in_=emb_d[:, :],
                        in_offset=IndirectOffsetOnAxis(ap=idts[tt][:, :1],
                                                       axis=0))
                    xgs.append(xg)
                return xgs

            def p1_transpose(st, xgs, xtc, tt, spread=False):
                for hk in range(2):
                    pt = ptr.tile([128, 128], f32, tag="pt")
                    nc.tensor.transpose(pt[:, :],
                                        xgs[tt][:, hk * 128:(hk + 1) * 128],
                                        ident_f[:, :])
                    if spread and hk == 1:
                        nc.vector.tensor_copy(
                            xtc[:, hk, tt * 128:(tt + 1) * 128], pt[:, :])
                    else:
                        nc.scalar.copy(xtc[:, hk, tt * 128:(tt + 1) * 128],
                                       pt[:, :])

            def p1_gates(st, c, xtc, m, spread=False):
                tok0 = c * CTOK
                pg = pg1.tile([128, CHUNK_T, 32], f32, tag="pg")
                for k in range(2):
                    nc.tensor.matmul(
                        pg[:, :, :], wih_sl(k, m), xtc[:, k, :],
                        start=(k == 0), stop=(k == 1 and m >= 2))
                if m < 2:  # z-gate: add BIGMASK * is_padded(token)
                    nc.tensor.matmul(
                        pg[:, :, :], wbig[:1, :], maskr[:1, tok0: tok0 + CTOK],
                        start=False, stop=True)
                csl = slice(c * CHUNK_T, (c + 1) * CHUNK_T)
                if m < 4:
                    dst = gzr[:, csl, M2SLOT[m], :]
                else:
                    dst = gg[:, csl, m - 4, :]
                if spread and m % 2 == 1:
                    nc.vector.tensor_scalar(dst, pg[:, :, :],
                                            bias_sb[:, m:m + 1], None, OP.add)
                else:
                    nc.scalar.add(dst, pg[:, :, :], bias_sb[:, m:m + 1])

            # piece schedule: chunk c's work spread over the 8 steps of the
            # window two chunks earlier. Returns closures to emit at step st.
            chunk_state = {}

            def emit_piece(c, st, spread=False):
                if c >= NCHUNK:
                    return
                if st == 0:
                    xtc = xtp.tile([128, 2, CTOK], bf16, tag="xtc", name="xtc")
                    chunk_state[c] = {"xgs": p1_gather(c), "xtc": xtc}
                cs = chunk_state[c]
                if st == 1:
                    p1_transpose(st, cs["xgs"], cs["xtc"], 0, spread)
                elif st == 2:
                    p1_transpose(st, cs["xgs"], cs["xtc"], 1, spread)
                elif st >= 3:
                    m = st - 3
                    p1_gates(st, c, cs["xtc"], m, spread)
                    if st == 7:
                        p1_gates(st, c, cs["xtc"], 5, spread)

            # startup: chunk 0 completes first, chunks 1-3 pipeline behind
            for st in range(8):
                emit_piece(0, st, spread=True)
            for c in range(1, 4):
                for st in range(8):
                    emit_piece(c, st, spread=True)

            # logits embedding slice, prefetched piecemeal during recurrence
            NV = (VSC + 511) // 512
            evb = const.tile([128, 2, NV * 512], bf16)

            def emit_evb(i):
                if i >= 2 * NV:
                    return
                j, k = divmod(i, 2)
                v0 = j * 512
                vw = min(512, VSC - v0)
                nc.sync.dma_start(
                    evb[:, k, j * 512: j * 512 + vw],
                    embt_d[k * 128:(k + 1) * 128, v0:v0 + vw])

            # ---- recurrence ----
            # h is kept split as h = us + ws (us = z*h_prev, ws = (1-z)*hh);
            # PE contracts Whh against both parts, so the critical path after
            # tanh is just the ws tensor op -> 4 r-gate ws-matmuls -> sigmoid.
            hf = hst.tile([128, 2, 32], f32, tag="hf")
            usb = hst.tile([128, 2, 32], bf16, tag="usb")
            wsb = hst.tile([128, 2, 32], bf16, tag="wsb")
            nc.vector.memset(hf[:, :, :], 0.0)
            nc.vector.memset(usb[:, :, :], 0.0)
            nc.vector.memset(wsb[:, :, :], 0.0)

            for t in range(T):
                st = t % CHUNK_T
                przr = pzr.tile([128, 4, 32], f32, tag="przr")
                prg = pgg.tile([128, 2, 32], f32, tag="prg")

                # PE: injects first (run during previous step's EW window)
                nc.tensor.matmul(przr[:, 0:4, :], ident[:, :], gzr[:, t, :, :],
                                 start=True, stop=False)
                nc.tensor.matmul(prg[:, :, :], ident[:, :], bhhg_bc[:, :, :],
                                 start=True, stop=False)
                # us-side MMs (usb ready mid-EW of step t-1)
                for s in range(2):
                    for k in range(2):
                        nc.tensor.matmul(
                            przr[:, s, :], whh_sl(k, 2 + s), usb[:, k, :],
                            start=False, stop=False)
                for s in range(2):
                    for k in range(2):
                        nc.tensor.matmul(
                            przr[:, 2 + s, :], whh_sl(k, s), usb[:, k, :],
                            start=False, stop=False)
                for s in range(2):
                    for k in range(2):
                        nc.tensor.matmul(
                            prg[:, s, :], whh_sl(k, 4 + s), usb[:, k, :],
                            start=False, stop=False)
                # ws-side MMs: r-gate slots first — they gate sigmoid(r)
                for s in range(2):
                    for k in range(2):
                        nc.tensor.matmul(
                            przr[:, s, :], whh_sl(k, 2 + s), wsb[:, k, :],
                            start=False, stop=(k == 1))
                for s in range(2):
                    for k in range(2):
                        nc.tensor.matmul(
                            przr[:, 2 + s, :], whh_sl(k, s), wsb[:, k, :],
                            start=False, stop=(k == 1))
                for s in range(2):
                    for k in range(2):
                        nc.tensor.matmul(
                            prg[:, s, :], whh_sl(k, 4 + s), wsb[:, k, :],
                            start=False, stop=(k == 1))

                # ACT: sigmoid(r) first, then sigmoid(z)
                rz = ew.tile([128, 4, 32], f32, tag="rz")
                nc.scalar.activation(rz[:, 0:2, :], przr[:, 0:2, :], AF.Sigmoid)
                nc.scalar.activation(rz[:, 2:4, :], przr[:, 2:4, :], AF.Sigmoid)

                # DVE critical: qs = r*prg ; q2 = qs + gg[t] (bf16: 2x DVE)
                qs = ew.tile([128, 2, 32], bf16, tag="qs")
                nc.vector.tensor_tensor(qs[:, :, :], rz[:, 0:2, :], prg[:, :, :],
                                        op=OP.mult)
                q2 = ew.tile([128, 2, 32], bf16, tag="q2")
                nc.vector.tensor_tensor(q2[:, :, :], qs[:, :, :], gg[:, t, :, :],
                                        op=OP.add)
                # DVE off-path (during tanh): zc = 1-z ; us = z*hf (bf16)
                zc = ew.tile([128, 2, 32], bf16, tag="zc")
                nc.vector.tensor_scalar(zc[:, :, :], rz[:, 2:4, :], -1.0, 1.0,
                                        OP.mult, OP.add)
                usb = hst.tile([128, 2, 32], bf16, tag="usb")
                nc.vector.tensor_tensor(usb[:, :, :], rz[:, 2:4, :],
                                        hf[:, :, :], op=OP.mult)

                # ACT: tanh
                hh = ew.tile([128, 2, 32], bf16, tag="hh")
                nc.scalar.activation(hh[:, :, :], q2[:, :, :], AF.Tanh)

                # DVE: ws = zc*hh (bf16, feeds PE) ; hf = us + ws (f32 state)
                wsb = hst.tile([128, 2, 32], bf16, tag="wsb")
                nc.vector.tensor_tensor(wsb[:, :, :], zc[:, :, :], hh[:, :, :],
                                        op=OP.mult)
                hf = hst.tile([128, 2, 32], f32, tag="hf")
                nc.vector.tensor_tensor(hf[:, :, :], usb[:, :, :],
                                        wsb[:, :, :], op=OP.add)

                # phase-1 piece for chunk t//8 + 4, deprioritized so the
                # scheduler keeps it out of the critical DVE/ACT sequences
                with tc.high_priority(offset=-5000):
                    emit_piece(t // CHUNK_T + 4, st)
                if t >= 16:
                    emit_evb(t - 16)

            # ---- logits: AllGather h, then [B,VSC] = h @ embT_slice ----
            hb16 = hst.tile([128, 2, 32], bf16, tag="hb16", name="hb16")
            nc.vector.tensor_tensor(hb16[:, :, :], usb[:, :, :], wsb[:, :, :],
                                    op=OP.add)
            cc_in = ldram.tile([128, 2 * 32], bf16)
            nc.sync.dma_start(cc_in[:, :], hb16[:, :, :])
            cc_out = ldram.tile([NCORES, 128, 2 * 32], bf16)
            nc.gpsimd.collective_compute(
                "AllGather",
                mybir.AluOpType.bypass,
                replica_groups=[list(range(NCORES))],
                ins=[cc_in.opt()],
                outs=[cc_out.opt()],
            )
            hall = const.tile([128, 2, NCORES, 32], bf16)  # [p, k, core, b]
            for r in range(NCORES):
                nc.sync.dma_start(
                    hall[:, :, r, :],
                    cc_out[r].rearrange("p (k b) -> p k b", k=2))

            # logits in groups of 4x512 columns; fat contiguous output DMAs
            for bt in range(2):
                for g in range((NV + 3) // 4):
                    jlo, jhi = g * 4, min(g * 4 + 4, NV)
                    lo = lop.tile([128, 2048], bf16, tag=f"lo{bt}", name="lo")
                    for j in range(jlo, jhi):
                        v0 = j * 512
                        vw = min(512, VSC - v0)
                        pl = pg1.tile([128, 512], f32, tag="pg", name="pl")
                        for k in range(2):
                            nc.tensor.matmul(
                                pl[:, :vw],
                                hall[:, k, bt * 4:(bt + 1) * 4, :],
                                evb[:, k, j * 512: j * 512 + vw],
                                start=(k == 0), stop=(k == 1))
                        dst = lo[:, (j - jlo) * 512:(j - jlo) * 512 + vw]
                        if j % 2 == 0:
                            nc.scalar.copy(dst, pl[:, :vw])
                        else:
                            nc.vector.tensor_copy(dst, pl[:, :vw])
                    gw = (jhi - jlo - 1) * 512 + min(512, VSC - (jhi - 1) * 512)
                    nc.sync.dma_start(
                        out_d[bt * 128:(bt + 1) * 128, jlo * 512: jlo * 512 + gw],
                        lo[:, :gw])

    nc.compile()
    return nc


def _prep_inputs(input_ids, lengths, emb, w_ih, w_hh, b_ih, b_hh):
    bfd = ml_dtypes.bfloat16
    emb32 = np.ascontiguousarray(emb.astype(np.float32))
    wih16 = w_ih.astype(bfd)
    whh16 = w_hh.astype(bfd)
    biasf = (b_ih + b_hh).astype(np.float32).copy()
    biasf[2 * H:] = b_ih[2 * H:]          # h-candidate: b_ih only (pre r-mult)
    biasf = biasf.reshape(3 * H, 1)
    bhhg = b_hh[2 * H:].astype(np.float32).reshape(H, 1)

    in_maps = []
    for c in range(NCORES):
        bs = slice(c * BL, (c + 1) * BL)
        ids_c = np.ascontiguousarray(
            input_ids[bs].T.reshape(NTOK, 1).astype(np.int32))   # t-major
        mask_c = (np.arange(T)[:, None] >= lengths[bs][None, :])  # [T, BL]
        mask_c = np.ascontiguousarray(
            mask_c.reshape(1, NTOK).astype(bfd))
        v0 = c * VS
        embt_c = np.ascontiguousarray(emb32[v0:v0 + VSC].T.astype(bfd))
        in_maps.append({
            "ids": ids_c,
            "maskrow": mask_c,
            "emb": emb32,
            "embt": embt_c,
            "wih": wih16,
            "whh": whh16,
            "biasf": biasf,
            "idm": np.eye(128, dtype=np.float32),
            "bhhg": bhhg,
        })
    return in_maps


def _run(in_maps, trace=False):
    from concourse.bass_utils import run_bass_kernel_spmd
    if "nc" not in _cache:
        _cache["nc"] = _build_nc()
    return run_bass_kernel_spmd(
        _cache["nc"], in_maps, core_ids=list(range(NCORES)), trace=trace)


def kernel(input_ids, lengths, emb, w_ih, w_hh, b_ih, b_hh, _trace=False):
    input_ids = np.asarray(input_ids)
    lengths = np.asarray(lengths)
    emb = np.asarray(emb, dtype=np.float32)
    w_ih = np.asarray(w_ih, dtype=np.float32)
    w_hh = np.asarray(w_hh, dtype=np.float32)
    b_ih = np.asarray(b_ih, dtype=np.float32)
    b_hh = np.asarray(b_hh, dtype=np.float32)

    in_maps = _prep_inputs(input_ids, lengths, emb, w_ih, w_hh, b_ih, b_hh)
    res = _run(in_maps, trace=_trace)
    outs = res.results if hasattr(res, "results") else res
    logits = np.empty((B, V + 1), np.float32)
    for c in range(NCORES):
        oc = outs[c]["out"].astype(np.float32)
        w = VSC if c == NCORES - 1 else VS
        logits[:, c * VS: c * VS + w] = oc[:, :w]
    if _trace:
        return logits, res
    return logits


# revision 57
# speedup vs baseline: 1.0112x; 1.0112x over previous
"""GRU4Rec Trainium2 kernel: 8-core SPMD, latency-optimized recurrence.

Sharding: data-parallel over batch (32 seqs/core); vocab-sharded tied-embedding
logits with an on-device AllGather of the final hidden state.

Recurrence critical path per step (the serial cycle):
  PE r-gate matmuls -> ACT sigmoid(r) -> DVE qs,q2 -> ACT tanh -> DVE ws,hb -> PE
All other work is pushed off that cycle:
  - x-side gates, b_hh(g) injected into PSUM by PE identity-matmuls, emitted
    ahead of the weight matmuls so they run during the previous step's EW.
  - sigma(z), zc=1-z, us=z*h run during the tanh window.
  - phase-1 chunk work (gather/transpose/gates for t+16..) is emitted one
    piece per step, alternating ACT/DVE for the psum->sbuf copies.
PSUM slot order [r0, r1, z0, z1] lets sigmoid(r) fire after only 4 weight MMs.
"""

import numpy as np
import ml_dtypes

B, T, H, V = 256, 200, 256, 50000
NCORES = 8
BL = B // NCORES          # 32 sequences per core
NTOK = BL * T             # 6400 tokens per core
VS = 6250                 # vocab stride per core
VSC = VS + 1              # per-core logits width (overlap of 1)
CHUNK_T = 8               # timesteps per phase-1 chunk
NCHUNK = T // CHUNK_T     # 25
CTOK = BL * CHUNK_T       # 256 tokens per chunk
BIGMASK = 60.0            # sigmoid(x + 60) == 1.0 in fp32

_cache = {}


def _build_nc(debug=False):
    import concourse.bass as bass
    import concourse.mybir as mybir
    import concourse.tile as tile
    from concourse import bacc
    from concourse.bass import IndirectOffsetOnAxis

    f32 = mybir.dt.float32
    bf16 = mybir.dt.bfloat16
    i32 = mybir.dt.int32
    AF = mybir.ActivationFunctionType
    OP = mybir.AluOpType

    nc = bacc.Bacc(None, target_bir_lowering=False, debug=False, num_devices=NCORES)

    ids_d = nc.dram_tensor("ids", [NTOK, 1], i32, kind="ExternalInput")
    maskr_d = nc.dram_tensor("maskrow", [1, NTOK], bf16, kind="ExternalInput")
    emb_d = nc.dram_tensor("emb", [V + 1, H], f32, kind="ExternalInput")
    embt_d = nc.dram_tensor("embt", [H, VSC], bf16, kind="ExternalInput")
    wih_d = nc.dram_tensor("wih", [H, 3 * H], bf16, kind="ExternalInput")
    whh_d = nc.dram_tensor("whh", [H, 3 * H], bf16, kind="ExternalInput")
    biasf_d = nc.dram_tensor("biasf", [3 * H, 1], f32, kind="ExternalInput")
    idm_d = nc.dram_tensor("idm", [128, 128], f32, kind="ExternalInput")
    bhhg_d = nc.dram_tensor("bhhg", [H, 1], f32, kind="ExternalInput")
    out_d = nc.dram_tensor("out", [B, VSC], bf16, kind="ExternalOutput")

    # weight-column m-slice -> psum slot: weights are [z0 z1 r0 r1 g0 g1],
    # psum zr-slots are [r0 r1 z0 z1]
    M2SLOT = {0: 2, 1: 3, 2: 0, 3: 1}

    with tile.TileContext(nc) as tc:
        with (
            tc.tile_pool(name="const", bufs=1) as const,
            tc.tile_pool(name="gstore", bufs=1) as gstore,
            tc.tile_pool(name="gin", bufs=6) as gin,
            tc.tile_pool(name="xtp", bufs=4) as xtp,
            tc.tile_pool(name="ew", bufs=3) as ew,
            tc.tile_pool(name="hst", bufs=3) as hst,
            tc.tile_pool(name="lop", bufs=2) as lop,
            tc.tile_pool(name="ldram", bufs=1, space="DRAM") as ldram,
            tc.tile_pool(name="ptr", bufs=1, space="PSUM") as ptr,
            tc.tile_pool(name="pg1", bufs=4, space="PSUM") as pg1,
            tc.tile_pool(name="pzr", bufs=2, space="PSUM") as pzr,
            tc.tile_pool(name="pgg", bufs=1, space="PSUM") as pgg,
        ):
            # ---- constants / weights ----
            wih_sb = const.tile([128, 2 * 3 * H], bf16)
            whh_sb = const.tile([128, 2 * 3 * H], bf16)
            ident_f = const.tile([128, 128], f32)
            nc.sync.dma_start(ident_f[:, :], idm_d[:, :])

            # ids for the 4 startup chunks go out before the bulk weight DMAs
            gin_pool = gin

            def p1_ids(c):
                tok0 = c * CTOK
                idts = []
                for tt in range(2):
                    idt = gin_pool.tile([128, 1], i32, tag="idt", name="idt")
                    nc.sync.dma_start(
                        idt[:, :],
                        ids_d[tok0 + tt * 128: tok0 + (tt + 1) * 128, :])
                    idts.append(idt)
                return idts

            start_idts = {0: p1_ids(0)}

            for k in range(2):
                nc.sync.dma_start(wih_sb[:, k * 768:(k + 1) * 768],
                                  wih_d[k * 128:(k + 1) * 128, :])
            start_idts.update({c: p1_ids(c) for c in range(1, 2)})
            for k in range(2):
                nc.sync.dma_start(whh_sb[:, k * 768:(k + 1) * 768],
                                  whh_d[k * 128:(k + 1) * 128, :])
            ident = const.tile([128, 128], bf16)
            nc.vector.tensor_copy(ident[:, :], ident_f[:, :])
            wbig = const.tile([1, 128], bf16)
            nc.vector.memset(wbig[:, :], BIGMASK)
            maskr = const.tile([1, NTOK], bf16)
            nc.sync.dma_start(maskr[:, :], maskr_d[:, :])
            bias_sb = const.tile([128, 6], f32)
            nc.sync.dma_start(bias_sb[:, :],
                              biasf_d.rearrange("(m p) o -> p (m o)", p=128))
            bhhg_sb = const.tile([128, 2], f32)
            nc.sync.dma_start(bhhg_sb[:, :],
                              bhhg_d.rearrange("(g p) o -> p (g o)", p=128))
            bhhg_bc = const.tile([128, 2, 32], bf16)
            for g in range(2):
                nc.vector.tensor_copy(bhhg_bc[:, g, :],
                                      bhhg_sb[:, g:g + 1].to_broadcast([128, 32]))

            # persistent x-side gate stores
            gzr = gstore.tile([128, T, 4, 32], bf16)   # slots [r0 r1 z0 z1]
            gg = gstore.tile([128, T, 2, 32], bf16)    # candidate x-gates

            def wih_sl(k, m):
                return wih_sb[:, k * 768 + m * 128: k * 768 + (m + 1) * 128]

            def whh_sl(k, m):
                return whh_sb[:, k * 768 + m * 128: k * 768 + (m + 1) * 128]

            # ---- phase-1 chunk pieces ----
            # xtc: [H-part, k, token] f32-gathered -> transposed -> bf16
            def p1_gather(c):
                idts = start_idts.pop(c, None) or p1_ids(c)
                xgs = []
                for tt in range(2):
                    xg = gin.tile([128, H], f32, tag=f"xg{tt}")
                    nc.gpsimd.indirect_dma_start(
                        out=xg[:, :], out_offset=None, in_=emb_d[:, :],
                        in_offset=IndirectOffsetOnAxis(ap=idts[tt][:, :1],
                                                       axis=0))
                    xgs.append(xg)
                return xgs

            def p1_transpose(st, xgs, xtc, tt, spread=False):
                for hk in range(2):
                    pt = ptr.tile([128, 128], f32, tag="pt")
                    nc.tensor.transpose(pt[:, :],
                                        xgs[tt][:, hk * 128:(hk + 1) * 128],
                                        ident_f[:, :])
                    if spread and hk == 1:
                        nc.vector.tensor_copy(
                            xtc[:, hk, tt * 128:(tt + 1) * 128], pt[:, :])
                    else:
                        nc.scalar.copy(xtc[:, hk, tt * 128:(tt + 1) * 128],
                                       pt[:, :])

            def p1_gates(st, c, xtc, m, spread=False):
                tok0 = c * CTOK
                pg = pg1.tile([128, CHUNK_T, 32], f32, tag="pg")
                for k in range(2):
                    nc.tensor.matmul(
                        pg[:, :, :], wih_sl(k, m), xtc[:, k, :],
                        start=(k == 0), stop=(k == 1 and m >= 2))
                if m < 2:  # z-gate: add BIGMASK * is_padded(token)
                    nc.tensor.matmul(
                        pg[:, :, :], wbig[:1, :], maskr[:1, tok0: tok0 + CTOK],
                        start=False, stop=True)
                csl = slice(c * CHUNK_T, (c + 1) * CHUNK_T)
                if m < 4:
                    dst = gzr[:, csl, M2SLOT[m], :]
                else:
                    dst = gg[:, csl, m - 4, :]
                if spread and m % 2 == 1:
                    nc.vector.tensor_scalar(dst, pg[:, :, :],
                                            bias_sb[:, m:m + 1], None, OP.add)
                else:
                    nc.scalar.add(dst, pg[:, :, :], bias_sb[:, m:m + 1])

            # piece schedule: chunk c's work spread over the 8 steps of the
            # window two chunks earlier. Returns closures to emit at step st.
            chunk_state = {}

            def emit_piece(c, st, spread=False):
                if c >= NCHUNK:
                    return
                if st == 0:
                    xtc = xtp.tile([128, 2, CTOK], bf16, tag="xtc", name="xtc")
                    chunk_state[c] = {"xgs": p1_gather(c), "xtc": xtc}
                cs = chunk_state[c]
                if st == 1:
                    p1_transpose(st, cs["xgs"], cs["xtc"], 0, spread)
                elif st == 2:
                    p1_transpose(st, cs["xgs"], cs["xtc"], 1, spread)
                elif st >= 3:
                    m = st - 3
                    p1_gates(st, c, cs["xtc"], m, spread)
                    if st == 7:
                        p1_gates(st, c, cs["xtc"], 5, spread)

            # startup: chunk 0 completes first, chunks 1-2 pipeline behind
            for st in range(8):
                emit_piece(0, st, spread=True)
            for c in range(1, 2):
                for st in range(8):
                    emit_piece(c, st, spread=True)

            # logits embedding slice, prefetched piecemeal during recurrence
            NV = (VSC + 511) // 512
            evb = const.tile([128, 2, NV * 512], bf16)

            def emit_evb(i):
                if i >= 2 * NV:
                    return
                j, k = divmod(i, 2)
                v0 = j * 512
                vw = min(512, VSC - v0)
                nc.sync.dma_start(
                    evb[:, k, j * 512: j * 512 + vw],
                    embt_d[k * 128:(k + 1) * 128, v0:v0 + vw])

            # ---- recurrence ----
            # h is kept split as h = us + ws (us = z*h_prev, ws = (1-z)*hh);
            # PE contracts Whh against both parts, so the critical path after
            # tanh is just the ws tensor op -> 4 r-gate ws-matmuls -> sigmoid.
            hf = hst.tile([128, 2, 32], f32, tag="hf")
            usb = hst.tile([128, 2, 32], bf16, tag="usb")
            wsb = hst.tile([128, 2, 32], bf16, tag="wsb")
            nc.vector.memset(hf[:, :, :], 0.0)
            nc.vector.memset(usb[:, :, :], 0.0)
            nc.vector.memset(wsb[:, :, :], 0.0)

            for t in range(T):
                st = t % CHUNK_T
                przr = pzr.tile([128, 4, 32], f32, tag="przr")
                prg = pgg.tile([128, 2, 32], f32, tag="prg")

                # PE: injects first (run during previous step's EW window)
                nc.tensor.matmul(przr[:, 0:4, :], ident[:, :], gzr[:, t, :, :],
                                 start=True, stop=False)
                nc.tensor.matmul(prg[:, :, :], ident[:, :], bhhg_bc[:, :, :],
                                 start=True, stop=False)
                # us-side MMs (usb ready mid-EW of step t-1)
                for s in range(2):
                    for k in range(2):
                        nc.tensor.matmul(
                            przr[:, s, :], whh_sl(k, 2 + s), usb[:, k, :],
                            start=False, stop=False)
                for s in range(2):
                    for k in range(2):
                        nc.tensor.matmul(
                            przr[:, 2 + s, :], whh_sl(k, s), usb[:, k, :],
                            start=False, stop=False)
                for s in range(2):
                    for k in range(2):
                        nc.tensor.matmul(
                            prg[:, s, :], whh_sl(k, 4 + s), usb[:, k, :],
                            start=False, stop=False)
                # ws-side MMs: r-gate slots first — they gate sigmoid(r)
                for s in range(2):
                    for k in range(2):
                        nc.tensor.matmul(
                            przr[:, s, :], whh_sl(k, 2 + s), wsb[:, k, :],
                            start=False, stop=(k == 1))
                for s in range(2):
                    for k in range(2):
                        nc.tensor.matmul(
                            przr[:, 2 + s, :], whh_sl(k, s), wsb[:, k, :],
                            start=False, stop=(k == 1))
                for s in range(2):
                    for k in range(2):
                        nc.tensor.matmul(
                            prg[:, s, :], whh_sl(k, 4 + s), wsb[:, k, :],
                            start=False, stop=(k == 1))

                # ACT: sigmoid(r) first, then sigmoid(z)
                rz = ew.tile([128, 4, 32], f32, tag="rz")
                nc.scalar.activation(rz[:, 0:2, :], przr[:, 0:2, :], AF.Sigmoid)
                nc.scalar.activation(rz[:, 2:4, :], przr[:, 2:4, :], AF.Sigmoid)

                # DVE critical: qs = r*prg ; q2 = qs + gg[t] (bf16: 2x DVE)
                qs = ew.tile([128, 2, 32], bf16, tag="qs")
                nc.vector.tensor_tensor(qs[:, :, :], rz[:, 0:2, :], prg[:, :, :],
                                        op=OP.mult)
                q2 = ew.tile([128, 2, 32], bf16, tag="q2")
                nc.vector.tensor_tensor(q2[:, :, :], qs[:, :, :], gg[:, t, :, :],
                                        op=OP.add)
                # DVE off-path (during tanh): zc = 1-z ; us = z*hf (bf16)
                zc = ew.tile([128, 2, 32], bf16, tag="zc")
                nc.vector.tensor_scalar(zc[:, :, :], rz[:, 2:4, :], -1.0, 1.0,
                                        OP.mult, OP.add)
                usb = hst.tile([128, 2, 32], bf16, tag="usb")
                nc.vector.tensor_tensor(usb[:, :, :], rz[:, 2:4, :],
                                        hf[:, :, :], op=OP.mult)

                # ACT: tanh
                hh = ew.tile([128, 2, 32], bf16, tag="hh")
                nc.scalar.activation(hh[:, :, :], q2[:, :, :], AF.Tanh)

                # DVE: ws = zc*hh (bf16, feeds PE) ; hf = us + ws (f32 state)
                wsb = hst.tile([128, 2, 32], bf16, tag="wsb")
                nc.vector.tensor_tensor(wsb[:, :, :], zc[:, :, :], hh[:, :, :],
                                        op=OP.mult)
                hf = hst.tile([128, 2, 32], f32, tag="hf")
                nc.vector.tensor_tensor(hf[:, :, :], usb[:, :, :],
                                        wsb[:, :, :], op=OP.add)

                # phase-1 piece for chunk t//8 + 4, deprioritized so the
                # scheduler keeps it out of the critical DVE/ACT sequences
                with tc.high_priority(offset=-5000):
                    emit_piece(t // CHUNK_T + 2, st)
                if t >= 16:
                    emit_evb(t - 16)

            # ---- logits: AllGather h, then [B,VSC] = h @ embT_slice ----
            hb16 = hst.tile([128, 2, 32], bf16, tag="hb16", name="hb16")
            nc.vector.tensor_tensor(hb16[:, :, :], usb[:, :, :], wsb[:, :, :],
                                    op=OP.add)
            cc_in = ldram.tile([128, 2 * 32], bf16)
            nc.sync.dma_start(cc_in[:, :], hb16[:, :, :])
            cc_out = ldram.tile([NCORES, 128, 2 * 32], bf16)
            nc.gpsimd.collective_compute(
                "AllGather",
                mybir.AluOpType.bypass,
                replica_groups=[list(range(NCORES))],
                ins=[cc_in.opt()],
                outs=[cc_out.opt()],
            )
            hall = const.tile([128, 2, NCORES, 32], bf16)  # [p, k, core, b]
            for r in range(NCORES):
                nc.sync.dma_start(
                    hall[:, :, r, :],
                    cc_out[r].rearrange("p (k b) -> p k b", k=2))

            # logits in groups of 4x512 columns; fat contiguous output DMAs
            for bt in range(2):
                for g in range((NV + 3) // 4):
                    jlo, jhi = g * 4, min(g * 4 + 4, NV)
                    lo = lop.tile([128, 2048], bf16, tag=f"lo{bt}", name="lo")
                    for j in range(jlo, jhi):
                        v0 = j * 512
                        vw = min(512, VSC - v0)
                        pl = pg1.tile([128, 512], f32, tag="pg", name="pl")
                        for k in range(2):
                            nc.tensor.matmul(
                                pl[:, :vw],
                                hall[:, k, bt * 4:(bt + 1) * 4, :],
                                evb[:, k, j * 512: j * 512 + vw],
                                start=(k == 0), stop=(k == 1))
                        dst = lo[:, (j - jlo) * 512:(j - jlo) * 512 + vw]
                        if j % 2 == 0:
                            nc.scalar.copy(dst, pl[:, :vw])
                        else:
                            nc.vector.tensor_copy(dst, pl[:, :vw])
                    gw = (jhi - jlo - 1) * 512 + min(512, VSC - (jhi - 1) * 512)
                    nc.sync.dma_start(
                        out_d[bt * 128:(bt + 1) * 128, jlo * 512: jlo * 512 + gw],
                        lo[:, :gw])

    nc.compile()
    return nc


def _prep_inputs(input_ids, lengths, emb, w_ih, w_hh, b_ih, b_hh):
    bfd = ml_dtypes.bfloat16
    emb32 = np.ascontiguousarray(emb.astype(np.float32))
    wih16 = w_ih.astype(bfd)
    whh16 = w_hh.astype(bfd)
    biasf = (b_ih + b_hh).astype(np.float32).copy()
    biasf[2 * H:] = b_ih[2 * H:]          # h-candidate: b_ih only (pre r-mult)
    biasf = biasf.reshape(3 * H, 1)
    bhhg = b_hh[2 * H:].astype(np.float32).reshape(H, 1)

    in_maps = []
    for c in range(NCORES):
        bs = slice(c * BL, (c + 1) * BL)
        ids_c = np.ascontiguousarray(
            input_ids[bs].T.reshape(NTOK, 1).astype(np.int32))   # t-major
        mask_c = (np.arange(T)[:, None] >= lengths[bs][None, :])  # [T, BL]
        mask_c = np.ascontiguousarray(
            mask_c.reshape(1, NTOK).astype(bfd))
        v0 = c * VS
        embt_c = np.ascontiguousarray(emb32[v0:v0 + VSC].T.astype(bfd))
        in_maps.append({
            "ids": ids_c,
            "maskrow": mask_c,
            "emb": emb32,
            "embt": embt_c,
            "wih": wih16,
            "whh": whh16,
            "biasf": biasf,
            "idm": np.eye(128, dtype=np.float32),
            "bhhg": bhhg,
        })
    return in_maps


def _run(in_maps, trace=False):
    from concourse.bass_utils import run_bass_kernel_spmd
    if "nc" not in _cache:
        _cache["nc"] = _build_nc()
    return run_bass_kernel_spmd(
        _cache["nc"], in_maps, core_ids=list(range(NCORES)), trace=trace)


def kernel(input_ids, lengths, emb, w_ih, w_hh, b_ih, b_hh, _trace=False):
    input_ids = np.asarray(input_ids)
    lengths = np.asarray(lengths)
    emb = np.asarray(emb, dtype=np.float32)
    w_ih = np.asarray(w_ih, dtype=np.float32)
    w_hh = np.asarray(w_hh, dtype=np.float32)
    b_ih = np.asarray(b_ih, dtype=np.float32)
    b_hh = np.asarray(b_hh, dtype=np.float32)

    in_maps = _prep_inputs(input_ids, lengths, emb, w_ih, w_hh, b_ih, b_hh)
    res = _run(in_maps, trace=_trace)
    outs = res.results if hasattr(res, "results") else res
    logits = np.empty((B, V + 1), np.float32)
    for c in range(NCORES):
        oc = outs[c]["out"].astype(np.float32)
        w = VSC if c == NCORES - 1 else VS
        logits[:, c * VS: c * VS + w] = oc[:, :w]
    if _trace:
        return logits, res
    return logits


# revision 58
# speedup vs baseline: 1.0178x; 1.0065x over previous
"""GRU4Rec Trainium2 kernel: 8-core SPMD, latency-optimized recurrence.

Sharding: data-parallel over batch (32 seqs/core); vocab-sharded tied-embedding
logits with an on-device AllGather of the final hidden state.

Recurrence critical path per step (the serial cycle):
  PE r-gate matmuls -> ACT sigmoid(r) -> DVE qs,q2 -> ACT tanh -> DVE ws,hb -> PE
All other work is pushed off that cycle:
  - x-side gates, b_hh(g) injected into PSUM by PE identity-matmuls, emitted
    ahead of the weight matmuls so they run during the previous step's EW.
  - sigma(z), zc=1-z, us=z*h run during the tanh window.
  - phase-1 chunk work (gather/transpose/gates for t+16..) is emitted one
    piece per step, alternating ACT/DVE for the psum->sbuf copies.
PSUM slot order [r0, r1, z0, z1] lets sigmoid(r) fire after only 4 weight MMs.
"""

import numpy as np
import ml_dtypes

B, T, H, V = 256, 200, 256, 50000
NCORES = 8
BL = B // NCORES          # 32 sequences per core
NTOK = BL * T             # 6400 tokens per core
VS = 6250                 # vocab stride per core
VSC = VS + 1              # per-core logits width (overlap of 1)
CHUNK_T = 8               # timesteps per phase-1 chunk
NCHUNK = T // CHUNK_T     # 25
CTOK = BL * CHUNK_T       # 256 tokens per chunk
BIGMASK = 60.0            # sigmoid(x + 60) == 1.0 in fp32

_cache = {}


def _build_nc(debug=False):
    import concourse.bass as bass
    import concourse.mybir as mybir
    import concourse.tile as tile
    from concourse import bacc
    from concourse.bass import IndirectOffsetOnAxis

    f32 = mybir.dt.float32
    bf16 = mybir.dt.bfloat16
    i32 = mybir.dt.int32
    AF = mybir.ActivationFunctionType
    OP = mybir.AluOpType

    nc = bacc.Bacc(None, target_bir_lowering=False, debug=False, num_devices=NCORES)

    ids_d = nc.dram_tensor("ids", [NTOK, 1], i32, kind="ExternalInput")
    maskr_d = nc.dram_tensor("maskrow", [1, NTOK], bf16, kind="ExternalInput")
    emb_d = nc.dram_tensor("emb", [V + 1, H], f32, kind="ExternalInput")
    embt_d = nc.dram_tensor("embt", [H, VSC], bf16, kind="ExternalInput")
    wih_d = nc.dram_tensor("wih", [H, 3 * H], bf16, kind="ExternalInput")
    whh_d = nc.dram_tensor("whh", [H, 3 * H], bf16, kind="ExternalInput")
    biasf_d = nc.dram_tensor("biasf", [3 * H, 1], f32, kind="ExternalInput")
    idm_d = nc.dram_tensor("idm", [128, 128], f32, kind="ExternalInput")
    bhhg_d = nc.dram_tensor("bhhg", [H, 1], f32, kind="ExternalInput")
    out_d = nc.dram_tensor("out", [B, VSC], bf16, kind="ExternalOutput")

    # weight-column m-slice -> psum slot: weights are [z0 z1 r0 r1 g0 g1],
    # psum zr-slots are [r0 r1 z0 z1]
    M2SLOT = {0: 2, 1: 3, 2: 0, 3: 1}

    with tile.TileContext(nc) as tc:
        with (
            tc.tile_pool(name="const", bufs=1) as const,
            tc.tile_pool(name="gstore", bufs=1) as gstore,
            tc.tile_pool(name="gin", bufs=6) as gin,
            tc.tile_pool(name="xtp", bufs=4) as xtp,
            tc.tile_pool(name="ew", bufs=3) as ew,
            tc.tile_pool(name="hst", bufs=3) as hst,
            tc.tile_pool(name="lop", bufs=4) as lop,
            tc.tile_pool(name="ldram", bufs=1, space="DRAM") as ldram,
            tc.tile_pool(name="ptr", bufs=1, space="PSUM") as ptr,
            tc.tile_pool(name="pg1", bufs=4, space="PSUM") as pg1,
            tc.tile_pool(name="pzr", bufs=2, space="PSUM") as pzr,
            tc.tile_pool(name="pgg", bufs=1, space="PSUM") as pgg,
        ):
            # ---- constants / weights ----
            wih_sb = const.tile([128, 2 * 3 * H], bf16)
            whh_sb = const.tile([128, 2 * 3 * H], bf16)
            ident_f = const.tile([128, 128], f32)
            nc.sync.dma_start(ident_f[:, :], idm_d[:, :])

            # ids for the 4 startup chunks go out before the bulk weight DMAs
            gin_pool = gin

            def p1_ids(c):
                tok0 = c * CTOK
                idts = []
                for tt in range(2):
                    idt = gin_pool.tile([128, 1], i32, tag="idt", name="idt")
                    nc.sync.dma_start(
                        idt[:, :],
                        ids_d[tok0 + tt * 128: tok0 + (tt + 1) * 128, :])
                    idts.append(idt)
                return idts

            start_idts = {0: p1_ids(0)}

            for k in range(2):
                nc.sync.dma_start(wih_sb[:, k * 768:(k + 1) * 768],
                                  wih_d[k * 128:(k + 1) * 128, :])
            start_idts.update({c: p1_ids(c) for c in range(1, 2)})
            for k in range(2):
                nc.sync.dma_start(whh_sb[:, k * 768:(k + 1) * 768],
                                  whh_d[k * 128:(k + 1) * 128, :])
            ident = const.tile([128, 128], bf16)
            nc.vector.tensor_copy(ident[:, :], ident_f[:, :])
            wbig = const.tile([1, 128], bf16)
            nc.vector.memset(wbig[:, :], BIGMASK)
            maskr = const.tile([1, NTOK], bf16)
            nc.sync.dma_start(maskr[:, :], maskr_d[:, :])
            bias_sb = const.tile([128, 6], f32)
            nc.sync.dma_start(bias_sb[:, :],
                              biasf_d.rearrange("(m p) o -> p (m o)", p=128))
            bhhg_sb = const.tile([128, 2], f32)
            nc.sync.dma_start(bhhg_sb[:, :],
                              bhhg_d.rearrange("(g p) o -> p (g o)", p=128))
            bhhg_bc = const.tile([128, 2, 32], bf16)
            for g in range(2):
                nc.vector.tensor_copy(bhhg_bc[:, g, :],
                                      bhhg_sb[:, g:g + 1].to_broadcast([128, 32]))

            # persistent x-side gate stores
            gzr = gstore.tile([128, T, 4, 32], bf16)   # slots [r0 r1 z0 z1]
            gg = gstore.tile([128, T, 2, 32], bf16)    # candidate x-gates

            def wih_sl(k, m):
                return wih_sb[:, k * 768 + m * 128: k * 768 + (m + 1) * 128]

            def whh_sl(k, m):
                return whh_sb[:, k * 768 + m * 128: k * 768 + (m + 1) * 128]

            # ---- phase-1 chunk pieces ----
            # xtc: [H-part, k, token] f32-gathered -> transposed -> bf16
            def p1_gather(c):
                idts = start_idts.pop(c, None) or p1_ids(c)
                xgs = []
                for tt in range(2):
                    xg = gin.tile([128, H], f32, tag=f"xg{tt}")
                    nc.gpsimd.indirect_dma_start(
                        out=xg[:, :], out_offset=None, in_=emb_d[:, :],
                        in_offset=IndirectOffsetOnAxis(ap=idts[tt][:, :1],
                                                       axis=0))
                    xgs.append(xg)
                return xgs

            def p1_transpose(st, xgs, xtc, tt, spread=False):
                for hk in range(2):
                    pt = ptr.tile([128, 128], f32, tag="pt")
                    nc.tensor.transpose(pt[:, :],
                                        xgs[tt][:, hk * 128:(hk + 1) * 128],
                                        ident_f[:, :])
                    if spread and hk == 1:
                        nc.vector.tensor_copy(
                            xtc[:, hk, tt * 128:(tt + 1) * 128], pt[:, :])
                    else:
                        nc.scalar.copy(xtc[:, hk, tt * 128:(tt + 1) * 128],
                                       pt[:, :])

            def p1_gates(st, c, xtc, m, spread=False):
                tok0 = c * CTOK
                pg = pg1.tile([128, CHUNK_T, 32], f32, tag="pg")
                for k in range(2):
                    nc.tensor.matmul(
                        pg[:, :, :], wih_sl(k, m), xtc[:, k, :],
                        start=(k == 0), stop=(k == 1 and m >= 2))
                if m < 2:  # z-gate: add BIGMASK * is_padded(token)
                    nc.tensor.matmul(
                        pg[:, :, :], wbig[:1, :], maskr[:1, tok0: tok0 + CTOK],
                        start=False, stop=True)
                csl = slice(c * CHUNK_T, (c + 1) * CHUNK_T)
                if m < 4:
                    dst = gzr[:, csl, M2SLOT[m], :]
                else:
                    dst = gg[:, csl, m - 4, :]
                if spread and m % 2 == 1:
                    nc.vector.tensor_scalar(dst, pg[:, :, :],
                                            bias_sb[:, m:m + 1], None, OP.add)
                else:
                    nc.scalar.add(dst, pg[:, :, :], bias_sb[:, m:m + 1])

            # piece schedule: chunk c's work spread over the 8 steps of the
            # window two chunks earlier. Returns closures to emit at step st.
            chunk_state = {}

            def emit_piece(c, st, spread=False):
                if c >= NCHUNK:
                    return
                if st == 0:
                    xtc = xtp.tile([128, 2, CTOK], bf16, tag="xtc", name="xtc")
                    chunk_state[c] = {"xgs": p1_gather(c), "xtc": xtc}
                cs = chunk_state[c]
                if st == 1:
                    p1_transpose(st, cs["xgs"], cs["xtc"], 0, spread)
                elif st == 2:
                    p1_transpose(st, cs["xgs"], cs["xtc"], 1, spread)
                elif st >= 3:
                    m = st - 3
                    p1_gates(st, c, cs["xtc"], m, spread)
                    if st == 7:
                        p1_gates(st, c, cs["xtc"], 5, spread)

            # startup: chunk 0 completes first, chunks 1-2 pipeline behind
            for st in range(8):
                emit_piece(0, st, spread=True)
            for c in range(1, 2):
                for st in range(8):
                    emit_piece(c, st, spread=True)

            # logits embedding slice, prefetched piecemeal during recurrence
            NV = (VSC + 511) // 512
            evb = const.tile([128, 2, NV * 512], bf16)

            def emit_evb(i):
                if i >= 2 * NV:
                    return
                j, k = divmod(i, 2)
                v0 = j * 512
                vw = min(512, VSC - v0)
                nc.sync.dma_start(
                    evb[:, k, j * 512: j * 512 + vw],
                    embt_d[k * 128:(k + 1) * 128, v0:v0 + vw])

            # ---- recurrence ----
            # h is kept split as h = us + ws (us = z*h_prev, ws = (1-z)*hh);
            # PE contracts Whh against both parts, so the critical path after
            # tanh is just the ws tensor op -> 4 r-gate ws-matmuls -> sigmoid.
            hf = hst.tile([128, 2, 32], f32, tag="hf")
            usb = hst.tile([128, 2, 32], bf16, tag="usb")
            wsb = hst.tile([128, 2, 32], bf16, tag="wsb")
            nc.vector.memset(hf[:, :, :], 0.0)
            nc.vector.memset(usb[:, :, :], 0.0)
            nc.vector.memset(wsb[:, :, :], 0.0)

            for t in range(T):
                st = t % CHUNK_T
                przr = pzr.tile([128, 4, 32], f32, tag="przr")
                prg = pgg.tile([128, 2, 32], f32, tag="prg")

                # PE: injects first (run during previous step's EW window)
                nc.tensor.matmul(przr[:, 0:4, :], ident[:, :], gzr[:, t, :, :],
                                 start=True, stop=False)
                nc.tensor.matmul(prg[:, :, :], ident[:, :], bhhg_bc[:, :, :],
                                 start=True, stop=False)
                # us-side MMs (usb ready mid-EW of step t-1)
                for s in range(2):
                    for k in range(2):
                        nc.tensor.matmul(
                            przr[:, s, :], whh_sl(k, 2 + s), usb[:, k, :],
                            start=False, stop=False)
                for s in range(2):
                    for k in range(2):
                        nc.tensor.matmul(
                            przr[:, 2 + s, :], whh_sl(k, s), usb[:, k, :],
                            start=False, stop=False)
                for s in range(2):
                    for k in range(2):
                        nc.tensor.matmul(
                            prg[:, s, :], whh_sl(k, 4 + s), usb[:, k, :],
                            start=False, stop=False)
                # ws-side MMs: r-gate slots first — they gate sigmoid(r)
                for s in range(2):
                    for k in range(2):
                        nc.tensor.matmul(
                            przr[:, s, :], whh_sl(k, 2 + s), wsb[:, k, :],
                            start=False, stop=(k == 1))
                for s in range(2):
                    for k in range(2):
                        nc.tensor.matmul(
                            przr[:, 2 + s, :], whh_sl(k, s), wsb[:, k, :],
                            start=False, stop=(k == 1))
                for s in range(2):
                    for k in range(2):
                        nc.tensor.matmul(
                            prg[:, s, :], whh_sl(k, 4 + s), wsb[:, k, :],
                            start=False, stop=(k == 1))

                # ACT: sigmoid(r) first, then sigmoid(z)
                rz = ew.tile([128, 4, 32], f32, tag="rz")
                nc.scalar.activation(rz[:, 0:2, :], przr[:, 0:2, :], AF.Sigmoid)
                nc.scalar.activation(rz[:, 2:4, :], przr[:, 2:4, :], AF.Sigmoid)

                # DVE critical: qs = r*prg ; q2 = qs + gg[t] (bf16: 2x DVE)
                qs = ew.tile([128, 2, 32], bf16, tag="qs")
                nc.vector.tensor_tensor(qs[:, :, :], rz[:, 0:2, :], prg[:, :, :],
                                        op=OP.mult)
                q2 = ew.tile([128, 2, 32], bf16, tag="q2")
                nc.vector.tensor_tensor(q2[:, :, :], qs[:, :, :], gg[:, t, :, :],
                                        op=OP.add)
                # DVE off-path (during tanh): zc = 1-z ; us = z*hf (bf16)
                zc = ew.tile([128, 2, 32], bf16, tag="zc")
                nc.vector.tensor_scalar(zc[:, :, :], rz[:, 2:4, :], -1.0, 1.0,
                                        OP.mult, OP.add)
                usb = hst.tile([128, 2, 32], bf16, tag="usb")
                nc.vector.tensor_tensor(usb[:, :, :], rz[:, 2:4, :],
                                        hf[:, :, :], op=OP.mult)

                # ACT: tanh
                hh = ew.tile([128, 2, 32], bf16, tag="hh")
                nc.scalar.activation(hh[:, :, :], q2[:, :, :], AF.Tanh)

                # DVE: ws = zc*hh (bf16, feeds PE) ; hf = us + ws (f32 state)
                wsb = hst.tile([128, 2, 32], bf16, tag="wsb")
                nc.vector.tensor_tensor(wsb[:, :, :], zc[:, :, :], hh[:, :, :],
                                        op=OP.mult)
                hf = hst.tile([128, 2, 32], f32, tag="hf")
                nc.vector.tensor_tensor(hf[:, :, :], usb[:, :, :],
                                        wsb[:, :, :], op=OP.add)

                # phase-1 piece for chunk t//8 + 4, deprioritized so the
                # scheduler keeps it out of the critical DVE/ACT sequences
                with tc.high_priority(offset=-5000):
                    emit_piece(t // CHUNK_T + 2, st)
                if t >= 16:
                    emit_evb(t - 16)

            # ---- logits: AllGather h, then [B,VSC] = h @ embT_slice ----
            hb16 = hst.tile([128, 2, 32], bf16, tag="hb16", name="hb16")
            nc.vector.tensor_tensor(hb16[:, :, :], usb[:, :, :], wsb[:, :, :],
                                    op=OP.add)
            cc_in = ldram.tile([128, 2 * 32], bf16)
            nc.sync.dma_start(cc_in[:, :], hb16[:, :, :])
            cc_out = ldram.tile([NCORES, 128, 2 * 32], bf16)
            nc.gpsimd.collective_compute(
                "AllGather",
                mybir.AluOpType.bypass,
                replica_groups=[list(range(NCORES))],
                ins=[cc_in.opt()],
                outs=[cc_out.opt()],
            )
            hall = const.tile([128, 2, NCORES, 32], bf16)  # [p, k, core, b]
            for r in range(NCORES):
                nc.sync.dma_start(
                    hall[:, :, r, :],
                    cc_out[r].rearrange("p (k b) -> p k b", k=2))

            # logits in groups of 4x512 columns; fat contiguous output DMAs
            for bt in range(2):
                for g in range((NV + 1) // 2):
                    jlo, jhi = g * 2, min(g * 2 + 2, NV)
                    lo = lop.tile([128, 1024], bf16, tag=f"lo{bt}", name="lo")
                    for j in range(jlo, jhi):
                        v0 = j * 512
                        vw = min(512, VSC - v0)
                        pl = pg1.tile([128, 512], f32, tag="pg", name="pl")
                        for k in range(2):
                            nc.tensor.matmul(
                                pl[:, :vw],
                                hall[:, k, bt * 4:(bt + 1) * 4, :],
                                evb[:, k, j * 512: j * 512 + vw],
                                start=(k == 0), stop=(k == 1))
                        dst = lo[:, (j - jlo) * 512:(j - jlo) * 512 + vw]
                        if j % 2 == 0:
                            nc.scalar.copy(dst, pl[:, :vw])
                        else:
                            nc.vector.tensor_copy(dst, pl[:, :vw])
                    gw = (jhi - jlo - 1) * 512 + min(512, VSC - (jhi - 1) * 512)
                    nc.sync.dma_start(
                        out_d[bt * 128:(bt + 1) * 128, jlo * 512: jlo * 512 + gw],
                        lo[:, :gw])

    nc.compile()
    return nc


def _prep_inputs(input_ids, lengths, emb, w_ih, w_hh, b_ih, b_hh):
    bfd = ml_dtypes.bfloat16
    emb32 = np.ascontiguousarray(emb.astype(np.float32))
    wih16 = w_ih.astype(bfd)
    whh16 = w_hh.astype(bfd)
    biasf = (b_ih + b_hh).astype(np.float32).copy()
    biasf[2 * H:] = b_ih[2 * H:]          # h-candidate: b_ih only (pre r-mult)
    biasf = biasf.reshape(3 * H, 1)
    bhhg = b_hh[2 * H:].astype(np.float32).reshape(H, 1)

    in_maps = []
    for c in range(NCORES):
        bs = slice(c * BL, (c + 1) * BL)
        ids_c = np.ascontiguousarray(
            input_ids[bs].T.reshape(NTOK, 1).astype(np.int32))   # t-major
        mask_c = (np.arange(T)[:, None] >= lengths[bs][None, :])  # [T, BL]
        mask_c = np.ascontiguousarray(
            mask_c.reshape(1, NTOK).astype(bfd))
        v0 = c * VS
        embt_c = np.ascontiguousarray(emb32[v0:v0 + VSC].T.astype(bfd))
        in_maps.append({
            "ids": ids_c,
            "maskrow": mask_c,
            "emb": emb32,
            "embt": embt_c,
            "wih": wih16,
            "whh": whh16,
            "biasf": biasf,
            "idm": np.eye(128, dtype=np.float32),
            "bhhg": bhhg,
        })
    return in_maps


def _run(in_maps, trace=False):
    from concourse.bass_utils import run_bass_kernel_spmd
    if "nc" not in _cache:
        _cache["nc"] = _build_nc()
    return run_bass_kernel_spmd(
        _cache["nc"], in_maps, core_ids=list(range(NCORES)), trace=trace)


def kernel(input_ids, lengths, emb, w_ih, w_hh, b_ih, b_hh, _trace=False):
    input_ids = np.asarray(input_ids)
    lengths = np.asarray(lengths)
    emb = np.asarray(emb, dtype=np.float32)
    w_ih = np.asarray(w_ih, dtype=np.float32)
    w_hh = np.asarray(w_hh, dtype=np.float32)
    b_ih = np.asarray(b_ih, dtype=np.float32)
    b_hh = np.asarray(b_hh, dtype=np.float32)

    in_maps = _prep_inputs(input_ids, lengths, emb, w_ih, w_hh, b_ih, b_hh)
    res = _run(in_maps, trace=_trace)
    outs = res.results if hasattr(res, "results") else res
    logits = np.empty((B, V + 1), np.float32)
    for c in range(NCORES):
        oc = outs[c]["out"].astype(np.float32)
        w = VSC if c == NCORES - 1 else VS
        logits[:, c * VS: c * VS + w] = oc[:, :w]
    if _trace:
        return logits, res
    return logits


# revision 59
# speedup vs baseline: 1.0266x; 1.0087x over previous
"""GRU4Rec Trainium2 kernel: 8-core SPMD, latency-optimized recurrence.

Sharding: data-parallel over batch (32 seqs/core); vocab-sharded tied-embedding
logits with an on-device AllGather of the final hidden state.

Recurrence critical path per step (the serial cycle):
  PE r-gate matmuls -> ACT sigmoid(r) -> DVE qs,q2 -> ACT tanh -> DVE ws,hb -> PE
All other work is pushed off that cycle:
  - x-side gates, b_hh(g) injected into PSUM by PE identity-matmuls, emitted
    ahead of the weight matmuls so they run during the previous step's EW.
  - sigma(z), zc=1-z, us=z*h run during the tanh window.
  - phase-1 chunk work (gather/transpose/gates for t+16..) is emitted one
    piece per step, alternating ACT/DVE for the psum->sbuf copies.
PSUM slot order [r0, r1, z0, z1] lets sigmoid(r) fire after only 4 weight MMs.
"""

import numpy as np
import ml_dtypes

B, T, H, V = 256, 200, 256, 50000
NCORES = 8
BL = B // NCORES          # 32 sequences per core
NTOK = BL * T             # 6400 tokens per core
VS = 6250                 # vocab stride per core
VSC = VS + 1              # per-core logits width (overlap of 1)
CHUNK_T = 8               # timesteps per phase-1 chunk
NCHUNK = T // CHUNK_T     # 25
CTOK = BL * CHUNK_T       # 256 tokens per chunk
BIGMASK = 60.0            # sigmoid(x + 60) == 1.0 in fp32

_cache = {}


def _build_nc(t_run=T, debug=False):
    import concourse.bass as bass
    import concourse.mybir as mybir
    import concourse.tile as tile
    from concourse import bacc
    from concourse.bass import IndirectOffsetOnAxis

    f32 = mybir.dt.float32
    bf16 = mybir.dt.bfloat16
    i32 = mybir.dt.int32
    AF = mybir.ActivationFunctionType
    OP = mybir.AluOpType

    nc = bacc.Bacc(None, target_bir_lowering=False, debug=False, num_devices=NCORES)

    ids_d = nc.dram_tensor("ids", [NTOK, 1], i32, kind="ExternalInput")
    maskr_d = nc.dram_tensor("maskrow", [1, NTOK], bf16, kind="ExternalInput")
    emb_d = nc.dram_tensor("emb", [V + 1, H], f32, kind="ExternalInput")
    embt_d = nc.dram_tensor("embt", [H, VSC], bf16, kind="ExternalInput")
    wih_d = nc.dram_tensor("wih", [H, 3 * H], bf16, kind="ExternalInput")
    whh_d = nc.dram_tensor("whh", [H, 3 * H], bf16, kind="ExternalInput")
    biasf_d = nc.dram_tensor("biasf", [3 * H, 1], f32, kind="ExternalInput")
    idm_d = nc.dram_tensor("idm", [128, 128], f32, kind="ExternalInput")
    bhhg_d = nc.dram_tensor("bhhg", [H, 1], f32, kind="ExternalInput")
    out_d = nc.dram_tensor("out", [B, VSC], bf16, kind="ExternalOutput")

    # weight-column m-slice -> psum slot: weights are [z0 z1 r0 r1 g0 g1],
    # psum zr-slots are [r0 r1 z0 z1]
    M2SLOT = {0: 2, 1: 3, 2: 0, 3: 1}

    with tile.TileContext(nc) as tc:
        with (
            tc.tile_pool(name="const", bufs=1) as const,
            tc.tile_pool(name="gstore", bufs=1) as gstore,
            tc.tile_pool(name="gin", bufs=6) as gin,
            tc.tile_pool(name="xtp", bufs=4) as xtp,
            tc.tile_pool(name="ew", bufs=3) as ew,
            tc.tile_pool(name="hst", bufs=3) as hst,
            tc.tile_pool(name="lop", bufs=4) as lop,
            tc.tile_pool(name="ldram", bufs=1, space="DRAM") as ldram,
            tc.tile_pool(name="ptr", bufs=1, space="PSUM") as ptr,
            tc.tile_pool(name="pg1", bufs=4, space="PSUM") as pg1,
            tc.tile_pool(name="pzr", bufs=2, space="PSUM") as pzr,
            tc.tile_pool(name="pgg", bufs=1, space="PSUM") as pgg,
        ):
            # ---- constants / weights ----
            wih_sb = const.tile([128, 2 * 3 * H], bf16)
            whh_sb = const.tile([128, 2 * 3 * H], bf16)
            ident_f = const.tile([128, 128], f32)
            nc.sync.dma_start(ident_f[:, :], idm_d[:, :])

            # ids for the 4 startup chunks go out before the bulk weight DMAs
            gin_pool = gin

            def p1_ids(c):
                tok0 = c * CTOK
                idts = []
                for tt in range(2):
                    idt = gin_pool.tile([128, 1], i32, tag="idt", name="idt")
                    nc.sync.dma_start(
                        idt[:, :],
                        ids_d[tok0 + tt * 128: tok0 + (tt + 1) * 128, :])
                    idts.append(idt)
                return idts

            start_idts = {0: p1_ids(0)}

            for k in range(2):
                nc.sync.dma_start(wih_sb[:, k * 768:(k + 1) * 768],
                                  wih_d[k * 128:(k + 1) * 128, :])
            start_idts.update({c: p1_ids(c) for c in range(1, 2)})
            for k in range(2):
                nc.sync.dma_start(whh_sb[:, k * 768:(k + 1) * 768],
                                  whh_d[k * 128:(k + 1) * 128, :])
            ident = const.tile([128, 128], bf16)
            nc.vector.tensor_copy(ident[:, :], ident_f[:, :])
            wbig = const.tile([1, 128], bf16)
            nc.vector.memset(wbig[:, :], BIGMASK)
            maskr = const.tile([1, NTOK], bf16)
            nc.sync.dma_start(maskr[:, :], maskr_d[:, :])
            bias_sb = const.tile([128, 6], f32)
            nc.sync.dma_start(bias_sb[:, :],
                              biasf_d.rearrange("(m p) o -> p (m o)", p=128))
            bhhg_sb = const.tile([128, 2], f32)
            nc.sync.dma_start(bhhg_sb[:, :],
                              bhhg_d.rearrange("(g p) o -> p (g o)", p=128))
            bhhg_bc = const.tile([128, 2, 32], bf16)
            for g in range(2):
                nc.vector.tensor_copy(bhhg_bc[:, g, :],
                                      bhhg_sb[:, g:g + 1].to_broadcast([128, 32]))

            # persistent x-side gate stores
            gzr = gstore.tile([128, T, 4, 32], bf16)   # slots [r0 r1 z0 z1]
            gg = gstore.tile([128, T, 2, 32], bf16)    # candidate x-gates

            def wih_sl(k, m):
                return wih_sb[:, k * 768 + m * 128: k * 768 + (m + 1) * 128]

            def whh_sl(k, m):
                return whh_sb[:, k * 768 + m * 128: k * 768 + (m + 1) * 128]

            # ---- phase-1 chunk pieces ----
            # xtc: [H-part, k, token] f32-gathered -> transposed -> bf16
            def p1_gather(c):
                idts = start_idts.pop(c, None) or p1_ids(c)
                xgs = []
                for tt in range(2):
                    xg = gin.tile([128, H], f32, tag=f"xg{tt}")
                    nc.gpsimd.indirect_dma_start(
                        out=xg[:, :], out_offset=None, in_=emb_d[:, :],
                        in_offset=IndirectOffsetOnAxis(ap=idts[tt][:, :1],
                                                       axis=0))
                    xgs.append(xg)
                return xgs

            def p1_transpose(st, xgs, xtc, tt, spread=False):
                for hk in range(2):
                    pt = ptr.tile([128, 128], f32, tag="pt")
                    nc.tensor.transpose(pt[:, :],
                                        xgs[tt][:, hk * 128:(hk + 1) * 128],
                                        ident_f[:, :])
                    if spread and hk == 1:
                        nc.vector.tensor_copy(
                            xtc[:, hk, tt * 128:(tt + 1) * 128], pt[:, :])
                    else:
                        nc.scalar.copy(xtc[:, hk, tt * 128:(tt + 1) * 128],
                                       pt[:, :])

            def p1_gates(st, c, xtc, m, spread=False):
                tok0 = c * CTOK
                pg = pg1.tile([128, CHUNK_T, 32], f32, tag="pg")
                for k in range(2):
                    nc.tensor.matmul(
                        pg[:, :, :], wih_sl(k, m), xtc[:, k, :],
                        start=(k == 0), stop=(k == 1 and m >= 2))
                if m < 2:  # z-gate: add BIGMASK * is_padded(token)
                    nc.tensor.matmul(
                        pg[:, :, :], wbig[:1, :], maskr[:1, tok0: tok0 + CTOK],
                        start=False, stop=True)
                csl = slice(c * CHUNK_T, (c + 1) * CHUNK_T)
                if m < 4:
                    dst = gzr[:, csl, M2SLOT[m], :]
                else:
                    dst = gg[:, csl, m - 4, :]
                if spread and m % 2 == 1:
                    nc.vector.tensor_scalar(dst, pg[:, :, :],
                                            bias_sb[:, m:m + 1], None, OP.add)
                else:
                    nc.scalar.add(dst, pg[:, :, :], bias_sb[:, m:m + 1])

            # piece schedule: chunk c's work spread over the 8 steps of the
            # window two chunks earlier. Returns closures to emit at step st.
            chunk_state = {}

            def emit_piece(c, st, spread=False):
                if c >= NCHUNK:
                    return
                if st == 0:
                    xtc = xtp.tile([128, 2, CTOK], bf16, tag="xtc", name="xtc")
                    chunk_state[c] = {"xgs": p1_gather(c), "xtc": xtc}
                cs = chunk_state[c]
                if st == 1:
                    p1_transpose(st, cs["xgs"], cs["xtc"], 0, spread)
                elif st == 2:
                    p1_transpose(st, cs["xgs"], cs["xtc"], 1, spread)
                elif st >= 3:
                    m = st - 3
                    p1_gates(st, c, cs["xtc"], m, spread)
                    if st == 7:
                        p1_gates(st, c, cs["xtc"], 5, spread)

            # startup: chunk 0 completes first, chunks 1-2 pipeline behind
            for st in range(8):
                emit_piece(0, st, spread=True)
            for c in range(1, 2):
                for st in range(8):
                    emit_piece(c, st, spread=True)

            # logits embedding slice, prefetched piecemeal during recurrence
            NV = (VSC + 511) // 512
            evb = const.tile([128, 2, NV * 512], bf16)

            def emit_evb(i):
                if i >= 2 * NV:
                    return
                j, k = divmod(i, 2)
                v0 = j * 512
                vw = min(512, VSC - v0)
                nc.sync.dma_start(
                    evb[:, k, j * 512: j * 512 + vw],
                    embt_d[k * 128:(k + 1) * 128, v0:v0 + vw])

            # ---- recurrence ----
            # h is kept split as h = us + ws (us = z*h_prev, ws = (1-z)*hh);
            # PE contracts Whh against both parts, so the critical path after
            # tanh is just the ws tensor op -> 4 r-gate ws-matmuls -> sigmoid.
            hf = hst.tile([128, 2, 32], f32, tag="hf")
            usb = hst.tile([128, 2, 32], bf16, tag="usb")
            wsb = hst.tile([128, 2, 32], bf16, tag="wsb")
            nc.vector.memset(hf[:, :, :], 0.0)
            nc.vector.memset(usb[:, :, :], 0.0)
            nc.vector.memset(wsb[:, :, :], 0.0)

            for t in range(t_run):
                st = t % CHUNK_T
                przr = pzr.tile([128, 4, 32], f32, tag="przr")
                prg = pgg.tile([128, 2, 32], f32, tag="prg")

                # PE: injects first (run during previous step's EW window)
                nc.tensor.matmul(przr[:, 0:4, :], ident[:, :], gzr[:, t, :, :],
                                 start=True, stop=False)
                nc.tensor.matmul(prg[:, :, :], ident[:, :], bhhg_bc[:, :, :],
                                 start=True, stop=False)
                # us-side MMs (usb ready mid-EW of step t-1)
                for s in range(2):
                    for k in range(2):
                        nc.tensor.matmul(
                            przr[:, s, :], whh_sl(k, 2 + s), usb[:, k, :],
                            start=False, stop=False)
                for s in range(2):
                    for k in range(2):
                        nc.tensor.matmul(
                            przr[:, 2 + s, :], whh_sl(k, s), usb[:, k, :],
                            start=False, stop=False)
                for s in range(2):
                    for k in range(2):
                        nc.tensor.matmul(
                            prg[:, s, :], whh_sl(k, 4 + s), usb[:, k, :],
                            start=False, stop=False)
                # ws-side MMs: r-gate slots first — they gate sigmoid(r)
                for s in range(2):
                    for k in range(2):
                        nc.tensor.matmul(
                            przr[:, s, :], whh_sl(k, 2 + s), wsb[:, k, :],
                            start=False, stop=(k == 1))
                for s in range(2):
                    for k in range(2):
                        nc.tensor.matmul(
                            przr[:, 2 + s, :], whh_sl(k, s), wsb[:, k, :],
                            start=False, stop=(k == 1))
                for s in range(2):
                    for k in range(2):
                        nc.tensor.matmul(
                            prg[:, s, :], whh_sl(k, 4 + s), wsb[:, k, :],
                            start=False, stop=(k == 1))

                # ACT: sigmoid(r) first, then sigmoid(z)
                rz = ew.tile([128, 4, 32], f32, tag="rz")
                nc.scalar.activation(rz[:, 0:2, :], przr[:, 0:2, :], AF.Sigmoid)
                nc.scalar.activation(rz[:, 2:4, :], przr[:, 2:4, :], AF.Sigmoid)

                # DVE critical: qs = r*prg ; q2 = qs + gg[t] (bf16: 2x DVE)
                qs = ew.tile([128, 2, 32], bf16, tag="qs")
                nc.vector.tensor_tensor(qs[:, :, :], rz[:, 0:2, :], prg[:, :, :],
                                        op=OP.mult)
                q2 = ew.tile([128, 2, 32], bf16, tag="q2")
                nc.vector.tensor_tensor(q2[:, :, :], qs[:, :, :], gg[:, t, :, :],
                                        op=OP.add)
                # DVE off-path (during tanh): zc = 1-z ; us = z*hf (bf16)
                zc = ew.tile([128, 2, 32], bf16, tag="zc")
                nc.vector.tensor_scalar(zc[:, :, :], rz[:, 2:4, :], -1.0, 1.0,
                                        OP.mult, OP.add)
                usb = hst.tile([128, 2, 32], bf16, tag="usb")
                nc.vector.tensor_tensor(usb[:, :, :], rz[:, 2:4, :],
                                        hf[:, :, :], op=OP.mult)

                # ACT: tanh
                hh = ew.tile([128, 2, 32], bf16, tag="hh")
                nc.scalar.activation(hh[:, :, :], q2[:, :, :], AF.Tanh)

                # DVE: ws = zc*hh (bf16, feeds PE) ; hf = us + ws (f32 state)
                wsb = hst.tile([128, 2, 32], bf16, tag="wsb")
                nc.vector.tensor_tensor(wsb[:, :, :], zc[:, :, :], hh[:, :, :],
                                        op=OP.mult)
                hf = hst.tile([128, 2, 32], f32, tag="hf")
                nc.vector.tensor_tensor(hf[:, :, :], usb[:, :, :],
                                        wsb[:, :, :], op=OP.add)

                # phase-1 piece for chunk t//8 + 4, deprioritized so the
                # scheduler keeps it out of the critical DVE/ACT sequences
                with tc.high_priority(offset=-5000):
                    emit_piece(t // CHUNK_T + 2, st)
                if t >= 16:
                    emit_evb(t - 16)

            # ---- logits: AllGather h, then [B,VSC] = h @ embT_slice ----
            hb16 = hst.tile([128, 2, 32], bf16, tag="hb16", name="hb16")
            nc.vector.tensor_tensor(hb16[:, :, :], usb[:, :, :], wsb[:, :, :],
                                    op=OP.add)
            cc_in = ldram.tile([128, 2 * 32], bf16)
            nc.sync.dma_start(cc_in[:, :], hb16[:, :, :])
            cc_out = ldram.tile([NCORES, 128, 2 * 32], bf16)
            nc.gpsimd.collective_compute(
                "AllGather",
                mybir.AluOpType.bypass,
                replica_groups=[list(range(NCORES))],
                ins=[cc_in.opt()],
                outs=[cc_out.opt()],
            )
            hall = const.tile([128, 2, NCORES, 32], bf16)  # [p, k, core, b]
            for r in range(NCORES):
                nc.sync.dma_start(
                    hall[:, :, r, :],
                    cc_out[r].rearrange("p (k b) -> p k b", k=2))

            # logits in groups of 4x512 columns; fat contiguous output DMAs
            for bt in range(2):
                for g in range((NV + 1) // 2):
                    jlo, jhi = g * 2, min(g * 2 + 2, NV)
                    lo = lop.tile([128, 1024], bf16, tag=f"lo{bt}", name="lo")
                    for j in range(jlo, jhi):
                        v0 = j * 512
                        vw = min(512, VSC - v0)
                        pl = pg1.tile([128, 512], f32, tag="pg", name="pl")
                        for k in range(2):
                            nc.tensor.matmul(
                                pl[:, :vw],
                                hall[:, k, bt * 4:(bt + 1) * 4, :],
                                evb[:, k, j * 512: j * 512 + vw],
                                start=(k == 0), stop=(k == 1))
                        dst = lo[:, (j - jlo) * 512:(j - jlo) * 512 + vw]
                        if j % 2 == 0:
                            nc.scalar.copy(dst, pl[:, :vw])
                        else:
                            nc.vector.tensor_copy(dst, pl[:, :vw])
                    gw = (jhi - jlo - 1) * 512 + min(512, VSC - (jhi - 1) * 512)
                    nc.sync.dma_start(
                        out_d[bt * 128:(bt + 1) * 128, jlo * 512: jlo * 512 + gw],
                        lo[:, :gw])

    nc.compile()
    return nc


def _prep_inputs(input_ids, lengths, emb, w_ih, w_hh, b_ih, b_hh):
    bfd = ml_dtypes.bfloat16
    emb32 = np.ascontiguousarray(emb.astype(np.float32))
    wih16 = w_ih.astype(bfd)
    whh16 = w_hh.astype(bfd)
    biasf = (b_ih + b_hh).astype(np.float32).copy()
    biasf[2 * H:] = b_ih[2 * H:]          # h-candidate: b_ih only (pre r-mult)
    biasf = biasf.reshape(3 * H, 1)
    bhhg = b_hh[2 * H:].astype(np.float32).reshape(H, 1)

    in_maps = []
    for c in range(NCORES):
        bs = slice(c * BL, (c + 1) * BL)
        ids_c = np.ascontiguousarray(
            input_ids[bs].T.reshape(NTOK, 1).astype(np.int32))   # t-major
        mask_c = (np.arange(T)[:, None] >= lengths[bs][None, :])  # [T, BL]
        mask_c = np.ascontiguousarray(
            mask_c.reshape(1, NTOK).astype(bfd))
        v0 = c * VS
        embt_c = np.ascontiguousarray(emb32[v0:v0 + VSC].T.astype(bfd))
        in_maps.append({
            "ids": ids_c,
            "maskrow": mask_c,
            "emb": emb32,
            "embt": embt_c,
            "wih": wih16,
            "whh": whh16,
            "biasf": biasf,
            "idm": np.eye(128, dtype=np.float32),
            "bhhg": bhhg,
        })
    return in_maps


def _run(in_maps, t_run, trace=False):
    from concourse.bass_utils import run_bass_kernel_spmd
    key = ("nc", t_run)
    if key not in _cache:
        _cache[key] = _build_nc(t_run)
    return run_bass_kernel_spmd(
        _cache[key], in_maps, core_ids=list(range(NCORES)), trace=trace)


def kernel(input_ids, lengths, emb, w_ih, w_hh, b_ih, b_hh, _trace=False):
    input_ids = np.asarray(input_ids)
    lengths = np.asarray(lengths)
    emb = np.asarray(emb, dtype=np.float32)
    w_ih = np.asarray(w_ih, dtype=np.float32)
    w_hh = np.asarray(w_hh, dtype=np.float32)
    b_ih = np.asarray(b_ih, dtype=np.float32)
    b_hh = np.asarray(b_hh, dtype=np.float32)

    in_maps = _prep_inputs(input_ids, lengths, emb, w_ih, w_hh, b_ih, b_hh)
    t_run = min(T, max(1, int(lengths.max())))
    res = _run(in_maps, t_run, trace=_trace)
    outs = res.results if hasattr(res, "results") else res
    logits = np.empty((B, V + 1), np.float32)
    for c in range(NCORES):
        oc = outs[c]["out"].astype(np.float32)
        w = VSC if c == NCORES - 1 else VS
        logits[:, c * VS: c * VS + w] = oc[:, :w]
    if _trace:
        return logits, res
    return logits
